# revision 1
# baseline (speedup 1.0000x reference)
"""Trainium2 Bass kernel for nn_MixtureOfBidders.

Strategy: pure data-parallel over tokens (8 cores x 512 tokens), all weights
replicated per core. On device, everything runs in a transposed layout
[feature partitions, token free-dim]:

  - confidence head + top-2 auction + softmax routing computed on device (fp32)
  - base SwiGLU gate/up matmuls computed once per token (fp32r)
  - per-expert LoRA-gate contribution added in PSUM via an identity-matmul
    trick (PE accumulates base + lora in one PSUM bank, so the vector engine
    never does the add for the gate path)
  - h_wsum = sum_e we_e * silu(g_e) * u_e accumulated in bf16
  - shared base_down matmul factored out of the expert loop (done once on
    h_wsum); per-expert down-LoRA uses one-shot PSUM matmuls + DVE accumulate
"""

import functools
import sys

import numpy as np

sys.path.insert(0, "/opt/trn_rl_repo")

import ml_dtypes  # noqa: E402

import concourse.bass as bass  # noqa: E402
from concourse import bacc  # noqa: E402
import concourse.mybir as mybir  # noqa: E402
import concourse.tile as tile  # noqa: E402
from concourse.bass_utils import run_bass_kernel_spmd  # noqa: E402

B, S, H, I, E, TOPK, R = 4, 1024, 2048, 7168, 8, 2, 64
SCALING = 16.0 / 64.0
N_CORES = 8
N_TOK = B * S  # 4096
T = N_TOK // N_CORES  # 512 tokens per core
HC = H // 128  # 16 contraction chunks over H
IT = int(__import__('os').environ.get('KIT', I // 128))  # chunks over I
KSTAGE = int(__import__('os').environ.get('KSTAGE', 4))

F32 = mybir.dt.float32
F32R = mybir.dt.float32r
BF16 = mybir.dt.bfloat16
BFNP = ml_dtypes.bfloat16
AF = mybir.ActivationFunctionType
OP = mybir.AluOpType


def r32(ap):
    return ap.bitcast(F32R)


def build_module() -> bass.Bass:
    nc = bacc.Bacc("TRN2", target_bir_lowering=False)

    # ---- dram I/O (per core) ----
    xT = nc.dram_tensor("xT", [H, T], F32R, kind="ExternalInput")
    conf_wt = nc.dram_tensor("conf_wt", [H, E], F32, kind="ExternalInput")
    conf_b = nc.dram_tensor("conf_b", [E, 1], F32, kind="ExternalInput")
    wealth = nc.dram_tensor("wealth", [E, 1], F32, kind="ExternalInput")
    guA = nc.dram_tensor("guA", [E, H, 2 * R], F32R, kind="ExternalInput")
    guB = nc.dram_tensor("guB", [E, 2 * R, I], BF16, kind="ExternalInput")
    bgate = nc.dram_tensor("bgate", [H, I], F32R, kind="ExternalInput")
    bup = nc.dram_tensor("bup", [H, I], F32R, kind="ExternalInput")
    bdown = nc.dram_tensor("bdown", [I, H], BF16, kind="ExternalInput")
    dA = nc.dram_tensor("dA", [E, I, R], BF16, kind="ExternalInput")
    dB = nc.dram_tensor("dB", [E, R, H], BF16, kind="ExternalInput")
    ident = nc.dram_tensor("ident", [128, 128], BF16, kind="ExternalInput")
    outT = nc.dram_tensor("outT", [H, T], F32, kind="ExternalOutput")

    with tile.TileContext(nc) as tc:
        with (
            tc.tile_pool(name="consts", bufs=1) as consts,
            tc.tile_pool(name="dram", bufs=1, space="DRAM") as dpool,
            tc.tile_pool(name="pw", bufs=6, space="PSUM") as pw,
            tc.tile_pool(name="ptd", bufs=2, space="PSUM") as ptd,
            tc.tile_pool(name="acc", bufs=IT) as accp,
            tc.tile_pool(name="td", bufs=E) as tdp,
            tc.tile_pool(name="xp", bufs=1) as xp,
            tc.tile_pool(name="tA", bufs=E) as tAp,
            tc.tile_pool(name="web", bufs=E) as webp,
        ):
            id_sb = consts.tile([128, 128], BF16)
            nc.sync.dma_start(out=id_sb, in_=ident[:, :])
            cb_sb = consts.tile([E, 1], F32)
            nc.sync.dma_start(out=cb_sb, in_=conf_b[:, :])
            wl_sb = consts.tile([E, 1], F32)
            nc.sync.dma_start(out=wl_sb, in_=wealth[:, :])

            acc_t = [
                accp.tile([128, T], BF16, tag="acc", name=f"acc{i}")
                for i in range(IT)
            ]
            td_t = [
                tdp.tile([64, T], BF16, tag="td", name=f"td{i}")
                for i in range(E)
            ]

            # ---------- load x ----------
            x_sb = xp.tile([128, HC, T], F32R)
            nc.sync.dma_start(
                out=x_sb, in_=xT[:, :].rearrange("(c p) t -> p c t", p=128)
            )

            # ---------- routing (fp32, scoped pool) ----------
            we_b = []
            with tc.tile_pool(name="rt", bufs=2) as rt:
                cw_sb = rt.tile([128, HC, E], F32, tag="cw")
                nc.sync.dma_start(
                    out=cw_sb,
                    in_=conf_wt[:, :].rearrange("(c p) e -> p c e", p=128),
                )
                p_cf = pw.tile([128, T], F32, tag="big")
                for hc in range(HC):
                    nc.tensor.matmul(
                        p_cf[0:E, :],
                        cw_sb[:, hc, :],
                        x_sb[:, hc, :].bitcast(F32),
                        start=(hc == 0),
                        stop=(hc == HC - 1),
                    )
                conf = rt.tile([E, T], F32, tag="conf")
                nc.scalar.activation(conf, p_cf[0:E, :], AF.Sigmoid, bias=cb_sb)
                bids = rt.tile([E, T], F32, tag="bids")
                nc.vector.tensor_scalar(bids, conf, wl_sb, None, op0=OP.mult)

                def pfold_max(src, n, name, dtag):
                    """max over pairs of rows: [n, T] -> [n//2, T] via DRAM bounce
                    (DVE partition offsets must be 32-aligned, so realign in DRAM)."""
                    half = n // 2
                    scr = dpool.tile([n, T], F32, tag="folds", name=name + "s")
                    nc.sync.dma_start(out=scr, in_=src)
                    t = rt.tile([half, 2, T], F32, tag="foldt", name=name + "t")
                    s_ap = scr[:, :]
                    bap = bass.AP(
                        tensor=s_ap.tensor,
                        offset=s_ap.offset,
                        ap=[[T, half], [half * T, 2], [1, T]],
                    )
                    nc.sync.dma_start(out=t, in_=bap)
                    dst = rt.tile([half, T], F32, tag=dtag, name=name + "d")
                    nc.vector.tensor_tensor(dst, t[:, 0, :], t[:, 1, :], op=OP.max)
                    return dst

                def pmax8(src, tag):
                    a = pfold_max(src, 8, tag + "a", "foldd")
                    b = pfold_max(a, 4, tag + "b", "foldd")
                    return pfold_max(b, 2, tag + "c", tag + "res")

                def bcast_rows(row_ap, nrows, tag):
                    """broadcast [1, T] fp32 row -> [nrows, T] via DRAM bounce."""
                    scr = dpool.tile([1, T], F32, tag="scr", name="scr_" + tag)
                    nc.sync.dma_start(out=scr, in_=row_ap)
                    dst = rt.tile([nrows, T], F32, tag="bc", name="bc_" + tag)
                    src = scr[0:1, :]
                    bap = bass.AP(
                        tensor=src.tensor,
                        offset=src.offset,
                        ap=[[0, nrows]] + list(src.ap[1:]),
                    )
                    nc.sync.dma_start(out=dst, in_=bap)
                    return dst

                m1 = pmax8(bids, "m1")
                m1b = bcast_rows(m1, E, "m1b")
                mask1 = rt.tile([E, T], F32, tag="mask1")
                nc.vector.tensor_tensor(mask1, bids, m1b, op=OP.is_equal)
                bids2 = rt.tile([E, T], F32, tag="bids2")
                nc.vector.scalar_tensor_tensor(
                    bids2, mask1, -1e6, bids, op0=OP.mult, op1=OP.add
                )
                m2 = pmax8(bids2, "m2")
                m2b = bcast_rows(m2, E, "m2b")
                mask2 = rt.tile([E, T], F32, tag="mask2")
                nc.vector.tensor_tensor(mask2, bids2, m2b, op=OP.is_equal)

                d12 = rt.tile([1, T], F32, tag="d12")
                nc.vector.tensor_sub(d12, m1, m2)
                w1 = rt.tile([1, T], F32, tag="w1")
                nc.scalar.activation(w1, d12, AF.Sigmoid)
                w2 = rt.tile([1, T], F32, tag="w2")
                nc.scalar.activation(w2, d12, AF.Sigmoid, scale=-1.0)
                w1b = bcast_rows(w1, E, "w1b")
                w2b = bcast_rows(w2, E, "w2b")
                wea = rt.tile([E, T], F32, tag="wea")
                nc.vector.tensor_mul(wea, mask1, w1b)
                web8 = rt.tile([E, T], F32, tag="web8")
                nc.vector.tensor_mul(web8, mask2, w2b)
                we8 = rt.tile([E, T], BF16, tag="we8")
                nc.vector.tensor_add(we8, wea, web8)

                # broadcast each expert's weight row to 128 partitions (bf16)
                scr_we = dpool.tile([E, T], BF16, tag="scrwe")
                nc.sync.dma_start(out=scr_we, in_=we8)
                for e in range(E):
                    wt = webp.tile([128, T], BF16, tag="web", name=f"web{e}")
                    src = scr_we[e : e + 1, :]
                    bap = bass.AP(
                        tensor=src.tensor,
                        offset=src.offset,
                        ap=[[0, 128]] + list(src.ap[1:]),
                    )
                    nc.sync.dma_start(out=wt, in_=bap)
                    we_b.append(wt)

            # ---------- main loop (scoped pools) ----------
            with (
                tc.tile_pool(name="wga", bufs=2) as wga,
                tc.tile_pool(name="wgw", bufs=2) as wgw,
                tc.tile_pool(name="wb", bufs=2) as wbp,
                tc.tile_pool(name="wdA", bufs=2) as wdAp,
                tc.tile_pool(name="bsb", bufs=2) as bsb,
                tc.tile_pool(name="ew", bufs=4) as ew,
            ):
                # tA = x @ [gate_A | up_A]  -> [128=(Rg|Ru), T] per expert
                tAgu = []
                for e in range(E if KSTAGE >= 2 else 0):
                    ga_sb = wga.tile([128, HC, 2 * R], F32R, tag="guA")
                    nc.sync.dma_start(
                        out=ga_sb,
                        in_=guA[e, :, :].rearrange("(c p) r -> p c r", p=128),
                    )
                    p_tA = pw.tile([128, T], F32, tag="big")
                    for hc in range(HC):
                        nc.tensor.matmul(
                            p_tA,
                            ga_sb[:, hc, :],
                            x_sb[:, hc, :],
                            start=(hc == 0),
                            stop=(hc == HC - 1),
                        )
                    tAg_sb = tAp.tile([64, T], BF16, tag="tAg", name=f"tAg{e}")
                    nc.scalar.copy(tAg_sb, p_tA[0:64, :])
                    tAu_sb = tAp.tile([64, T], BF16, tag="tAu", name=f"tAu{e}")
                    nc.scalar.copy(tAu_sb, p_tA[64:128, :])
                    tAgu.append((tAg_sb, tAu_sb))

                for it in range(IT if KSTAGE >= 3 else 0):
                    bg_w = wgw.tile([128, HC, 128], F32R, tag="bgw")
                    nc.sync.dma_start(
                        out=bg_w,
                        in_=bgate[:, it * 128 : (it + 1) * 128].rearrange(
                            "(c p) i -> p c i", p=128
                        ),
                    )
                    bu_w = wgw.tile([128, HC, 128], F32R, tag="buw")
                    nc.sync.dma_start(
                        out=bu_w,
                        in_=bup[:, it * 128 : (it + 1) * 128].rearrange(
                            "(c p) i -> p c i", p=128
                        ),
                    )
                    p_bg = pw.tile([128, T], F32, tag="big")
                    p_bu = pw.tile([128, T], F32, tag="big")
                    for hc in range(HC):
                        nc.tensor.matmul(
                            p_bg,
                            bg_w[:, hc, :],
                            x_sb[:, hc, :],
                            start=(hc == 0),
                            stop=(hc == HC - 1),
                        )
                    for hc in range(HC):
                        nc.tensor.matmul(
                            p_bu,
                            bu_w[:, hc, :],
                            x_sb[:, hc, :],
                            start=(hc == 0),
                            stop=(hc == HC - 1),
                        )
                    bg_s = bsb.tile([128, T], BF16, tag="bgs")
                    nc.scalar.copy(bg_s, p_bg)
                    bu_s = bsb.tile([128, T], BF16, tag="bus")
                    nc.scalar.copy(bu_s, p_bu)

                    guB_s = wbp.tile([64, E, 2, 128], BF16, tag="guB")
                    nc.sync.dma_start(
                        out=guB_s,
                        in_=guB[:, :, it * 128 : (it + 1) * 128].rearrange(
                            "e (gu r) i -> r e gu i", gu=2
                        ),
                    )
                    dA_s = wdAp.tile([128, E, R], BF16, tag="dA")
                    nc.sync.dma_start(
                        out=dA_s,
                        in_=dA[:, it * 128 : (it + 1) * 128, :].rearrange(
                            "e p r -> p e r"
                        ),
                    )

                    p_td = None
                    for e in range(E):
                        # g_e = base_g + lora_g in PSUM (identity-matmul trick)
                        p_g = pw.tile([128, T], F32, tag="big")
                        nc.tensor.matmul(p_g, id_sb, bg_s, start=True, stop=False)
                        nc.tensor.matmul(
                            p_g,
                            guB_s[:, e, 0, :],
                            tAgu[e][0],
                            start=False,
                            stop=True,
                        )
                        # lora_u alone in PSUM
                        p_lu = pw.tile([128, T], F32, tag="big")
                        nc.tensor.matmul(
                            p_lu,
                            guB_s[:, e, 1, :],
                            tAgu[e][1],
                            start=True,
                            stop=True,
                        )
                        sg = ew.tile([128, T], BF16, tag="sg")
                        nc.scalar.activation(sg, p_g, AF.Silu)
                        u_t = ew.tile([128, T], BF16, tag="u")
                        nc.vector.scalar_tensor_tensor(
                            u_t, p_lu, 1.0, bu_s, op0=OP.bypass, op1=OP.add
                        )
                        h_t = ew.tile([128, T], BF16, tag="h")
                        nc.vector.tensor_mul(h_t, sg, u_t)
                        if e == 0:
                            hw_t = acc_t[it]
                            nc.vector.tensor_mul(hw_t, h_t, we_b[e])
                        else:
                            hw_t = ew.tile([128, T], BF16, tag="hw")
                            nc.vector.tensor_mul(hw_t, h_t, we_b[e])
                            nc.vector.tensor_add(acc_t[it], acc_t[it], hw_t)
                        # down-lora partial: td[e] += hw_e @ dA[e]
                        p_td = ptd.tile([64, T], F32, tag="ptd")
                        nc.tensor.matmul(
                            p_td, dA_s[:, e, :], hw_t, start=True, stop=True
                        )
                        if it == 0:
                            nc.vector.tensor_copy(td_t[e], p_td)
                        else:
                            nc.vector.tensor_add(td_t[e], td_t[e], p_td)

            # ---------- down projection ----------
            with (
                tc.tile_pool(name="wd", bufs=2) as wd,
                tc.tile_pool(name="wdB", bufs=2) as wdB,
                tc.tile_pool(name="osb", bufs=3) as osb,
            ):
                dB4 = dB[:, :, :].rearrange("e r h -> r e h")
                for hc in range(HC if KSTAGE >= 4 else 0):
                    bd_s = wd.tile([128, IT, 128], BF16, tag="bd")
                    nc.sync.dma_start(
                        out=bd_s,
                        in_=bdown[0 : IT * 128, hc * 128 : (hc + 1) * 128].rearrange(
                            "(c p) h -> p c h", p=128
                        ),
                    )
                    dB_s = wdB.tile([64, E, 128], BF16, tag="dB")
                    nc.sync.dma_start(
                        out=dB_s, in_=dB4[:, :, hc * 128 : (hc + 1) * 128]
                    )
                    p_o = pw.tile([128, T], F32, tag="big")
                    for it in range(IT):
                        nc.tensor.matmul(
                            p_o, bd_s[:, it, :], acc_t[it], start=(it == 0), stop=False
                        )
                    for e in range(E):
                        nc.tensor.matmul(
                            p_o, dB_s[:, e, :], td_t[e], start=False, stop=(e == E - 1)
                        )
                    o_s = osb.tile([128, T], F32, tag="o")
                    nc.scalar.copy(o_s, p_o)
                    nc.sync.dma_start(
                        out=outT[hc * 128 : (hc + 1) * 128, :], in_=o_s
                    )
    nc.compile()
    return nc


@functools.lru_cache(maxsize=1)
def _get_module():
    return build_module()


def _host_prep(inputs):
    f32 = np.float32
    x = np.ascontiguousarray(np.asarray(inputs["hidden_states"], f32)).reshape(
        N_TOK, H
    )
    gate_A = np.asarray(inputs["gate_A"], f32)
    gate_B = np.asarray(inputs["gate_B"], f32)
    up_A = np.asarray(inputs["up_A"], f32)
    up_B = np.asarray(inputs["up_B"], f32)
    down_A = np.asarray(inputs["down_A"], f32)
    down_B = np.asarray(inputs["down_B"], f32)

    shared = {
        "conf_wt": np.ascontiguousarray(np.asarray(inputs["conf_W"], f32).T),
        "conf_b": np.ascontiguousarray(
            np.asarray(inputs["conf_b"], f32).reshape(E, 1)
        ),
        "wealth": np.ascontiguousarray(
            np.asarray(inputs["expert_wealth"], f32).reshape(E, 1)
        ),
        "guA": np.ascontiguousarray(np.concatenate([gate_A, up_A], axis=2)),
        "guB": np.ascontiguousarray(
            (np.concatenate([gate_B, up_B], axis=1) * f32(SCALING)).astype(BFNP)
        ),
        "bgate": np.ascontiguousarray(np.asarray(inputs["base_gate"], f32)),
        "bup": np.ascontiguousarray(np.asarray(inputs["base_up"], f32)),
        "bdown": np.ascontiguousarray(
            np.asarray(inputs["base_down"], f32).astype(BFNP)
        ),
        "dA": np.ascontiguousarray(down_A.astype(BFNP)),
        "dB": np.ascontiguousarray((down_B * f32(SCALING)).astype(BFNP)),
        "ident": np.eye(128, dtype=BFNP),
    }
    in_maps = []
    for c in range(N_CORES):
        m = dict(shared)
        m["xT"] = np.ascontiguousarray(x[c * T : (c + 1) * T, :].T)
        in_maps.append(m)
    return in_maps


def kernel(**inputs) -> np.ndarray:
    nc = _get_module()
    in_maps = _host_prep(inputs)
    res = run_bass_kernel_spmd(nc, in_maps, core_ids=list(range(N_CORES)))
    parts = [np.asarray(r["outT"], np.float32).T for r in res.results]
    return np.concatenate(parts, axis=0).reshape(B, S, H)



# revision 16
# speedup vs baseline: 1.2033x; 1.2033x over previous
"""Trainium2 Bass kernel for nn_MixtureOfBidders.

Data-parallel over tokens (8 cores x 512 tokens), weights replicated.
On-device layout is transposed: [feature partitions, token free-dim].

v3 design (vs v1 baseline at 1.63 ms):
  - all heavy matmuls in bf16 (fp32r is ~1.5x slower on the PE and blocks
    fast weight load); conf/auction stays fp32 to match reference top-k
  - weights pre-packed on host chunk-major so every weight DMA is one
    fully-contiguous block per partition with a small (<=32 KiB)
    partition stride (large partition strides corrupt DMA on HW)
  - routing (top-2 auction) via PE transposes + free-dim reductions in
    token-major layout -- no DRAM round-trips
  - per-expert combine weights broadcast to 128 partitions with one-hot
    matmuls instead of DRAM-bounce DMAs
  - down-LoRA rank partials accumulate in PSUM across all I-chunks
    (2 experts col-packed per bank; opened once by a zeroing matmul)
  - g = base + lora computed via identity-matmul accumulation in PSUM
    (PE-side add); u-path add runs ACT copy + DVE bf16 add
"""

import functools
import os
import sys

import numpy as np

sys.path.insert(0, "/opt/trn_rl_repo")

import ml_dtypes  # noqa: E402

import concourse.bass as bass  # noqa: E402
from concourse import bacc  # noqa: E402
import concourse.mybir as mybir  # noqa: E402
import concourse.tile as tile  # noqa: E402
from concourse.bass_utils import run_bass_kernel_spmd  # noqa: E402

B, S, H, I, E, TOPK, R = 4, 1024, 2048, 7168, 8, 2, 64
SCALING = 16.0 / 64.0
N_CORES = 8
N_TOK = B * S  # 4096
T = N_TOK // N_CORES  # 512 tokens per core
TC = T // 128  # 4 token chunks for transposed routing
HC = H // 128  # 16 contraction chunks over H
IT = I // 128  # 56 chunks over I
IT2 = IT // 2
HC2 = HC // 2
E2 = E // 2
TDPSUM = os.environ.get("TDPSUM", "1") == "1"

F32 = mybir.dt.float32
F32R = mybir.dt.float32r
BF16 = mybir.dt.bfloat16
BFNP = ml_dtypes.bfloat16
AF = mybir.ActivationFunctionType
OP = mybir.AluOpType
AX = mybir.AxisListType


def build_module() -> bass.Bass:
    nc = bacc.Bacc("TRN2", target_bir_lowering=False)

    # ---- dram I/O (per core) ----
    xT32 = nc.dram_tensor("xT32", [H, T], F32R, kind="ExternalInput")
    xTb = nc.dram_tensor("xTb", [H, T], BF16, kind="ExternalInput")
    conf_wt = nc.dram_tensor("conf_wt", [H, E], F32, kind="ExternalInput")
    conf_b = nc.dram_tensor("conf_b", [E, 1], F32, kind="ExternalInput")
    wealth = nc.dram_tensor("wealth", [E, 1], F32, kind="ExternalInput")
    # packed weights (see _host_prep for layouts); leading dim is the
    # chunk index so each load has a small partition stride
    guA = nc.dram_tensor("guA", [128, HC, E, 128], BF16, kind="ExternalInput")
    guBg = nc.dram_tensor("guBg", [IT2, 64, 2, E, 128], BF16, kind="ExternalInput")
    guBu = nc.dram_tensor("guBu", [IT2, 64, 2, E, 128], BF16, kind="ExternalInput")
    bgp = nc.dram_tensor("bgp", [IT2, 128, 2, HC, 128], BF16, kind="ExternalInput")
    bup = nc.dram_tensor("bup", [IT2, 128, 2, HC, 128], BF16, kind="ExternalInput")
    bdp = nc.dram_tensor("bdp", [HC2, 128, 2, IT, 128], BF16, kind="ExternalInput")
    dAp = nc.dram_tensor("dAp", [IT2, 128, 2, E, R], BF16, kind="ExternalInput")
    dBp = nc.dram_tensor("dBp", [HC2, 64, 2, E, 128], BF16, kind="ExternalInput")
    ident = nc.dram_tensor("ident", [128, 128], BF16, kind="ExternalInput")
    idf = nc.dram_tensor("idf", [128, 128], F32, kind="ExternalInput")
    bcast8 = nc.dram_tensor("bcast8", [E, E, 128], BF16, kind="ExternalInput")
    outT = nc.dram_tensor("outT", [H, T], F32, kind="ExternalOutput")

    with tile.TileContext(nc) as tc:
        with (
            tc.tile_pool(name="consts", bufs=1) as consts,
            tc.tile_pool(name="acc", bufs=IT) as accp,
            tc.tile_pool(name="xp", bufs=1) as xp,
            tc.tile_pool(name="tA", bufs=E) as tAp,
            tc.tile_pool(name="web", bufs=E) as webp,
            tc.tile_pool(name="tds", bufs=E) as tdsp,
        ):
            id_sb = consts.tile([128, 128], BF16)
            nc.sync.dma_start(out=id_sb, in_=ident[:, :])
            idf_sb = consts.tile([128, 128], F32)
            nc.sync.dma_start(out=idf_sb, in_=idf[:, :])
            bc8_sb = consts.tile([E, E, 128], BF16)
            nc.sync.dma_start(out=bc8_sb, in_=bcast8[:, :, :])
            cb_sb = consts.tile([E, 1], F32)
            nc.sync.dma_start(out=cb_sb, in_=conf_b[:, :])
            wl_sb = consts.tile([E, 1], F32)
            nc.sync.dma_start(out=wl_sb, in_=wealth[:, :])

            acc_t = [
                accp.tile([128, T], BF16, tag="acc", name=f"acc{i}")
                for i in range(IT)
            ]

            # ---------- load x (bf16 for compute) ----------
            xb_sb = xp.tile([128, HC, T], BF16)
            nc.sync.dma_start(
                out=xb_sb, in_=xTb[:, :].rearrange("(c p) t -> p c t", p=128)
            )

            we_b = []
            tAg = []
            tAu = []
            with (
                tc.tile_pool(name="rt", bufs=2) as rt,
                tc.tile_pool(name="xf", bufs=2) as xf,
                tc.tile_pool(name="wga", bufs=1) as wga,
                tc.tile_pool(name="prt", bufs=4, space="PSUM") as prt,
                tc.tile_pool(name="prs", bufs=2, space="PSUM") as prs,
            ):
                # ---------- confidence head (fp32, matches reference) ------
                cw_sb = rt.tile([128, HC, E], F32, tag="cw")
                nc.sync.dma_start(
                    out=cw_sb,
                    in_=conf_wt[:, :].rearrange("(c p) e -> p c e", p=128),
                )
                p_cf = prt.tile([128, T], F32, tag="pbig")
                for hc in range(HC):
                    xf_c = xf.tile([128, T], F32R, tag="xf")
                    nc.sync.dma_start(
                        out=xf_c, in_=xT32[hc * 128 : (hc + 1) * 128, :]
                    )
                    nc.tensor.matmul(
                        p_cf[0:E, :],
                        cw_sb[:, hc, :],
                        xf_c.bitcast(F32),
                        start=(hc == 0),
                        stop=(hc == HC - 1),
                    )
                conf = rt.tile([E, T], F32, tag="conf")
                nc.scalar.activation(conf, p_cf[0:E, :], AF.Sigmoid, bias=cb_sb)
                bids = rt.tile([E, T], F32, tag="bids")
                nc.vector.tensor_scalar(bids, conf, wl_sb, None, op0=OP.mult)

                # ---------- top-2 auction in token-major layout ----------
                # transpose bids into [128 tokens, E] chunks, reduce over the
                # free dim, build masks with per-partition scalars, transpose
                # the combine weights back.  No DRAM round trips.
                bT = rt.tile([128, TC, E], F32, tag="bT")
                m1 = rt.tile([128, TC], F32, tag="m1")
                m2 = rt.tile([128, TC], F32, tag="m2")
                msk1 = rt.tile([128, TC, E], F32, tag="msk1")
                msk2 = rt.tile([128, TC, E], F32, tag="msk2")
                b2 = rt.tile([128, TC, E], F32, tag="b2")
                weT = rt.tile([128, TC, E], F32, tag="weT")
                we8 = rt.tile([E, T], BF16, tag="we8")
                for c in range(TC):
                    p_bT = prs.tile([128, E], F32, tag="ptr")
                    nc.tensor.transpose(
                        p_bT, bids[:, c * 128 : (c + 1) * 128], idf_sb[0:E, 0:E]
                    )
                    nc.vector.tensor_copy(bT[:, c, :], p_bT)
                    nc.vector.tensor_reduce(
                        m1[:, c : c + 1], bT[:, c, :], AX.X, OP.max
                    )
                    nc.vector.tensor_scalar(
                        msk1[:, c, :], bT[:, c, :], m1[:, c : c + 1], None,
                        op0=OP.is_equal,
                    )
                    nc.vector.scalar_tensor_tensor(
                        b2[:, c, :], msk1[:, c, :], -1e6, bT[:, c, :],
                        op0=OP.mult, op1=OP.add,
                    )
                    nc.vector.tensor_reduce(
                        m2[:, c : c + 1], b2[:, c, :], AX.X, OP.max
                    )
                    nc.vector.tensor_scalar(
                        msk2[:, c, :], b2[:, c, :], m2[:, c : c + 1], None,
                        op0=OP.is_equal,
                    )
                d12 = rt.tile([128, TC], F32, tag="d12")
                nc.vector.tensor_sub(d12, m1, m2)
                w1 = rt.tile([128, TC], F32, tag="w1")
                nc.scalar.activation(w1, d12, AF.Sigmoid)
                w2 = rt.tile([128, TC], F32, tag="w2")
                nc.scalar.activation(w2, d12, AF.Sigmoid, scale=-1.0)
                for c in range(TC):
                    t1 = rt.tile([128, E], F32, tag="t1")
                    nc.vector.tensor_scalar(
                        t1, msk1[:, c, :], w1[:, c : c + 1], None, op0=OP.mult
                    )
                    t2 = rt.tile([128, E], F32, tag="t2")
                    nc.vector.tensor_scalar(
                        t2, msk2[:, c, :], w2[:, c : c + 1], None, op0=OP.mult
                    )
                    nc.vector.tensor_add(weT[:, c, :], t1, t2)
                    p_weT = prs.tile([E, 128], F32, tag="ptrb")
                    nc.tensor.transpose(p_weT, weT[:, c, :], idf_sb)
                    nc.vector.tensor_copy(
                        we8[:, c * 128 : (c + 1) * 128], p_weT
                    )
                # broadcast each expert's weight row to 128 partitions via
                # one-hot matmuls
                for e in range(E):
                    p_web = prt.tile([128, T], F32, tag="pbig")
                    nc.tensor.matmul(
                        p_web, bc8_sb[:, e, :], we8, start=True, stop=True
                    )
                    wt = webp.tile([128, T], BF16, tag="web", name=f"web{e}")
                    nc.scalar.copy(wt, p_web)
                    we_b.append(wt)

                # ---------- tA = x @ [gate_A | up_A]: split 64-row tiles ----
                ga_sb = wga.tile([128, HC, E, 128], BF16, tag="guA")
                nc.sync.dma_start(out=ga_sb, in_=guA[:, :, :, :])
                for e in range(E):
                    p_tA = prt.tile([128, T], F32, tag="pbig")
                    for hc in range(HC):
                        nc.tensor.matmul(
                            p_tA,
                            ga_sb[:, hc, e, :],
                            xb_sb[:, hc, :],
                            start=(hc == 0),
                            stop=(hc == HC - 1),
                        )
                    tg = tAp.tile([64, T], BF16, tag="tAg", name=f"tAg{e}")
                    nc.scalar.copy(tg, p_tA[0:64, :])
                    tAg.append(tg)
                    tu = tAp.tile([64, T], BF16, tag="tAu", name=f"tAu{e}")
                    nc.scalar.copy(tu, p_tA[64:128, :])
                    tAu.append(tu)

            # ---------- main loop over I chunk-pairs ----------
            with (
                tc.tile_pool(name="wgw", bufs=2) as wgw,
                tc.tile_pool(name="wb", bufs=2) as wbp,
                tc.tile_pool(name="wdA", bufs=2) as wdAp,
                tc.tile_pool(name="bsb", bufs=3) as bsb,
                tc.tile_pool(name="ew", bufs=3) as ew,
                tc.tile_pool(name="ptd", bufs=E2, space="PSUM") as ptd,
                tc.tile_pool(name="pw", bufs=4, space="PSUM") as pw,
            ):
                if TDPSUM:
                    # open each down-LoRA PSUM bank once with a zeroing
                    # matmul (sets has_written across all 128 partitions);
                    # the per-expert dA matmuls then accumulate with
                    # start=False in their own partition halves.
                    zro = bsb.tile([128, 128], BF16, tag="zro")
                    nc.vector.memset(zro, 0)
                    p_td = [
                        ptd.tile([128, T], F32, tag="ptd", name=f"ptd{p}")
                        for p in range(E2)
                    ]
                    for p in range(E2):
                        nc.tensor.matmul(
                            p_td[p],
                            zro,
                            xb_sb[:, 0, :],
                            start=True,
                            stop=False,
                            skip_group_check=True,
                        )
                else:
                    td_bf = [
                        tdsp.tile([64, T], BF16, tag="tds", name=f"tds{q}")
                        for q in range(E)
                    ]
                for it2 in range(IT2):
                    bg_w = wgw.tile([128, 2, HC, 128], BF16, tag="bgw")
                    nc.sync.dma_start(out=bg_w, in_=bgp[it2, :, :, :, :])
                    bu_w = wgw.tile([128, 2, HC, 128], BF16, tag="buw")
                    nc.sync.dma_start(out=bu_w, in_=bup[it2, :, :, :, :])
                    gBg_s = wbp.tile([64, 2, E, 128], BF16, tag="gBg")
                    nc.sync.dma_start(out=gBg_s, in_=guBg[it2, :, :, :, :])
                    gBu_s = wbp.tile([64, 2, E, 128], BF16, tag="gBu")
                    nc.sync.dma_start(out=gBu_s, in_=guBu[it2, :, :, :, :])
                    dA_s = wdAp.tile([128, 2, E, R], BF16, tag="dA")
                    nc.sync.dma_start(out=dA_s, in_=dAp[it2, :, :, :, :])

                    for j in range(2):
                        it = 2 * it2 + j
                        p_bg = pw.tile([128, T], F32, tag="big")
                        p_bu = pw.tile([128, T], F32, tag="big")
                        for hc in range(HC):
                            nc.tensor.matmul(
                                p_bg,
                                bg_w[:, j, hc, :],
                                xb_sb[:, hc, :],
                                start=(hc == 0),
                                stop=(hc == HC - 1),
                            )
                        for hc in range(HC):
                            nc.tensor.matmul(
                                p_bu,
                                bu_w[:, j, hc, :],
                                xb_sb[:, hc, :],
                                start=(hc == 0),
                                stop=(hc == HC - 1),
                            )
                        bg_s = bsb.tile([128, T], BF16, tag="bgs")
                        nc.scalar.copy(bg_s, p_bg)
                        bu_s = bsb.tile([128, T], BF16, tag="bus")
                        nc.scalar.copy(bu_s, p_bu)

                        for e in range(E):
                            # g = base_g + lora_g in PSUM (identity trick)
                            p_g = pw.tile([128, T], F32, tag="big")
                            nc.tensor.matmul(
                                p_g, id_sb, bg_s, start=True, stop=False
                            )
                            nc.tensor.matmul(
                                p_g,
                                gBg_s[:, j, e, :],
                                tAg[e],
                                start=False,
                                stop=True,
                            )
                            p_lu = pw.tile([128, T], F32, tag="big")
                            nc.tensor.matmul(
                                p_lu,
                                gBu_s[:, j, e, :],
                                tAu[e],
                                start=True,
                                stop=True,
                            )
                            sg = ew.tile([128, T], BF16, tag="sg")
                            nc.scalar.activation(sg, p_g, AF.Silu)
                            # lora_u: PSUM -> SBUF bf16 on ACT (own ports),
                            # then the add runs in DVE 2x mode
                            lu_s = ew.tile([128, T], BF16, tag="lu")
                            nc.scalar.copy(lu_s, p_lu)
                            u_t = ew.tile([128, T], BF16, tag="u")
                            nc.vector.tensor_add(u_t, lu_s, bu_s)
                            h_t = ew.tile([128, T], BF16, tag="h")
                            nc.vector.tensor_mul(h_t, sg, u_t)
                            if e == 0:
                                hw_t = acc_t[it]
                                nc.vector.tensor_mul(hw_t, h_t, we_b[e])
                            else:
                                hw_t = ew.tile([128, T], BF16, tag="hw")
                                nc.vector.tensor_mul(hw_t, h_t, we_b[e])
                                nc.vector.tensor_add(acc_t[it], acc_t[it], hw_t)
                            if TDPSUM:
                                nc.tensor.matmul(
                                    p_td[e // 2][
                                        (e % 2) * 64 : (e % 2) * 64 + 64, :
                                    ],
                                    dA_s[:, j, e, :],
                                    hw_t,
                                    start=False,
                                    stop=(it == IT - 1 and e == E - 1),
                                    skip_group_check=True,
                                )
                            else:
                                p_t1 = pw.tile([64, T], F32, tag="ptd1")
                                nc.tensor.matmul(
                                    p_t1, dA_s[:, j, e, :], hw_t,
                                    start=True, stop=True,
                                )
                                if it == 0:
                                    nc.vector.tensor_copy(td_bf[e], p_t1)
                                else:
                                    nc.vector.tensor_add(
                                        td_bf[e], td_bf[e], p_t1
                                    )

                # td: PSUM halves -> per-expert [64, T] bf16 tiles (base 0)
                if TDPSUM:
                    td_sb = []
                    for e in range(E):
                        ts = tdsp.tile([64, T], BF16, tag="tds", name=f"tds{e}")
                        nc.scalar.copy(
                            ts,
                            p_td[e // 2][(e % 2) * 64 : (e % 2) * 64 + 64, :],
                        )
                        td_sb.append(ts)
                else:
                    td_sb = td_bf

            # ---------- down projection ----------
            with (
                tc.tile_pool(name="wd", bufs=2) as wd,
                tc.tile_pool(name="wdB", bufs=2) as wdB,
                tc.tile_pool(name="osb", bufs=3) as osb,
                tc.tile_pool(name="po", bufs=2, space="PSUM") as pop,
            ):
                for hc2 in range(HC2):
                    bd_s = wd.tile([128, 2, IT, 128], BF16, tag="bd")
                    nc.sync.dma_start(out=bd_s, in_=bdp[hc2, :, :, :, :])
                    dB_s = wdB.tile([64, 2, E, 128], BF16, tag="dB")
                    nc.sync.dma_start(out=dB_s, in_=dBp[hc2, :, :, :, :])
                    for j in range(2):
                        hc = 2 * hc2 + j
                        p_o = pop.tile([128, T], F32, tag="po")
                        for it in range(IT):
                            nc.tensor.matmul(
                                p_o,
                                bd_s[:, j, it, :],
                                acc_t[it],
                                start=(it == 0),
                                stop=False,
                            )
                        for e in range(E):
                            nc.tensor.matmul(
                                p_o,
                                dB_s[:, j, e, :],
                                td_sb[e],
                                start=False,
                                stop=(e == E - 1),
                            )
                        o_s = osb.tile([128, T], F32, tag="o")
                        nc.scalar.copy(o_s, p_o)
                        nc.sync.dma_start(
                            out=outT[hc * 128 : (hc + 1) * 128, :], in_=o_s
                        )
    nc.compile()
    return nc


@functools.lru_cache(maxsize=1)
def _get_module():
    return build_module()


def _host_prep(inputs):
    f32 = np.float32
    x = np.ascontiguousarray(np.asarray(inputs["hidden_states"], f32)).reshape(
        N_TOK, H
    )
    gate_A = np.asarray(inputs["gate_A"], f32)
    gate_B = np.asarray(inputs["gate_B"], f32)
    up_A = np.asarray(inputs["up_A"], f32)
    up_B = np.asarray(inputs["up_B"], f32)
    down_A = np.asarray(inputs["down_A"], f32)
    down_B = np.asarray(inputs["down_B"], f32)
    bgate = np.asarray(inputs["base_gate"], f32)
    bup_w = np.asarray(inputs["base_up"], f32)
    bdown = np.asarray(inputs["base_down"], f32)

    # guA: [128, HC, E, 128]; [p, hc, e, r] = concat(A)[e, hc*128+p, r]
    guA_c = np.concatenate([gate_A, up_A], axis=2)  # [E, H, 2R]
    guAp = np.ascontiguousarray(
        guA_c.reshape(E, HC, 128, 2 * R).transpose(2, 1, 0, 3).astype(BFNP)
    )

    # guBg/guBu: [IT2, 64, 2, E, 128] (scaled)
    def pack_guB(w):  # [E, R, I]
        w = w * f32(SCALING)
        return np.ascontiguousarray(
            w.reshape(E, R, IT2, 2, 128).transpose(2, 1, 3, 0, 4).astype(BFNP)
        )

    guBgp = pack_guB(gate_B)
    guBup = pack_guB(up_B)

    # base gate/up: [IT2, 128, 2, HC, 128];
    # [it2, p, j, hc, i] = W[hc*128+p, (2*it2+j)*128+i]
    def pack_base(w):  # [H, I]
        return np.ascontiguousarray(
            w.reshape(HC, 128, IT2, 2, 128).transpose(2, 1, 3, 0, 4).astype(BFNP)
        )

    bgpk = pack_base(bgate)
    bupk = pack_base(bup_w)
    # base down: [HC2, 128, 2, IT, 128];
    # [hc2, p, j, it, h] = W[it*128+p, (2*hc2+j)*128+h]
    bdpk = np.ascontiguousarray(
        bdown.reshape(IT, 128, HC2, 2, 128).transpose(2, 1, 3, 0, 4).astype(BFNP)
    )
    # down_A: [IT2, 128, 2, E, R]
    dApk = np.ascontiguousarray(
        down_A.reshape(E, IT2, 2, 128, R).transpose(1, 3, 2, 0, 4).astype(BFNP)
    )
    # down_B: [HC2, 64, 2, E, 128] (scaled)
    dBpk = np.ascontiguousarray(
        (down_B * f32(SCALING))
        .reshape(E, R, HC2, 2, 128)
        .transpose(2, 1, 3, 0, 4)
        .astype(BFNP)
    )
    bc8 = np.zeros((E, E, 128), dtype=BFNP)
    for e in range(E):
        bc8[e, e, :] = BFNP(1.0)

    shared = {
        "conf_wt": np.ascontiguousarray(np.asarray(inputs["conf_W"], f32).T),
        "conf_b": np.ascontiguousarray(
            np.asarray(inputs["conf_b"], f32).reshape(E, 1)
        ),
        "wealth": np.ascontiguousarray(
            np.asarray(inputs["expert_wealth"], f32).reshape(E, 1)
        ),
        "guA": guAp,
        "guBg": guBgp,
        "guBu": guBup,
        "bgp": bgpk,
        "bup": bupk,
        "bdp": bdpk,
        "dAp": dApk,
        "dBp": dBpk,
        "ident": np.eye(128, dtype=BFNP),
        "idf": np.eye(128, dtype=f32),
        "bcast8": bc8,
    }
    in_maps = []
    for c in range(N_CORES):
        m = dict(shared)
        xc = np.ascontiguousarray(x[c * T : (c + 1) * T, :].T)
        m["xT32"] = xc
        m["xTb"] = np.ascontiguousarray(xc.astype(BFNP))
        in_maps.append(m)
    return in_maps


def kernel(**inputs) -> np.ndarray:
    nc = _get_module()
    in_maps = _host_prep(inputs)
    res = run_bass_kernel_spmd(nc, in_maps, core_ids=list(range(N_CORES)))
    parts = [np.asarray(r["outT"], np.float32).T for r in res.results]
    return np.concatenate(parts, axis=0).reshape(B, S, H)
